# revision 37
# baseline (speedup 1.0000x reference)
"""Multi-head causal attention (B=2, T=2048, C=2048, 16 heads, fp32) on 8
Trainium2 NeuronCores.

Sharding: data-parallel over batch (2) x tensor-parallel over heads
(4 heads/core).  Core c handles batch c//4, heads 4*(c%4)..4*(c%4)+3.
Each core computes q/k/v projections for its heads, causal softmax
attention, and a partial output projection (its heads' rows of Wout);
the host sums the 4 partials per batch.

Design notes:
  * all matmul operands in fp16 (PSUM accumulation stays fp32).  fp16's
    10-bit mantissa keeps max-rel error ~5e-4 (measured, vs 2e-2 gate);
    every value fits fp16 range (|scores| <~ 8, exp <~ 1.2e3, denom
    <~ 9e3 << 65504).  Halves DMA + SBUF so q^T/k^T stay RESIDENT in
    SBUF -- no DRAM bounce, no attention-phase input DMA.  fp16 matmuls
    run 1 PE cycle/row at ANY free width, and fp16 tensor ops get the
    DVE 2x mode.  (fp8 was simulated: 2x PE on paper, but even
    out-proj-only fp8e4m3 measures 3.8e-2 max-rel -- over the gate.)
  * softmax denominators don't burn a PE matmul per k-tile: exp tiles
    are accumulated on the DVE into an fp16 acc tile, and ONE
    ones-matmul per (head, block) partition-reduces + broadcasts it.
  * causal diagonal tiles are trimmed: scores matmul / exp / AV matmul
    only cover columns >= the diagonal.  Each (head, block) visits its
    off-diagonal (full-width, long-resident k/v) tiles first -- so the
    first AV matmul is full width (clean PSUM start) -- then the
    trimmed diagonal group.
  * attention runs TWO heads interleaved per j-step (independent
    score->exp->mask->AV chains, pending-delayed one step) so the
    in-order engine queues always hold independent work.
  * the whole kernel is software-pipelined: projection slab s+1's
    matmuls and the out-projection of block b-1 are PUMPED between the
    j-steps of attention block b as dependency-free PE filler while ACT
    chews the exp backlog.  PSUM pools are shared across phases (qk
    chains + scores; v chains + out-proj + denominators) to fit 8 banks.
  * out-proj PSUM drains ride the DVE; reciprocals use
    reciprocal_approx_fast (~5x cheaper, 18 good bits).
"""

import itertools

import numpy as np

import concourse.bass as bass
import concourse.tile as tile
from concourse import bacc, mybir
from concourse.bass_utils import run_bass_kernel_spmd

B, T, C = 2, 2048, 2048
H, DH = 16, 128
HPC = 4            # heads per core
KO = C // 128      # 16 contraction tiles
NSLAB = 4          # 512-wide t slabs in the projection phase
SLAB = T // NSLAB  # 512
NB = 4             # 512-wide tq blocks in attention
BW = T // NB       # 512
NT = T // 128      # 16 t tiles
SCALE = DH ** -0.5
F32 = mybir.dt.float32
FP16 = mybir.dt.float16


def build_nc():
    nc = bacc.Bacc("TRN2", target_bir_lowering=False, debug=False, num_devices=8)
    xt_d = nc.dram_tensor("xt", [C, T], FP16, kind="ExternalInput")
    wqk_d = nc.dram_tensor("wqk", [C, 2 * HPC * DH], FP16, kind="ExternalInput")
    wv_d = nc.dram_tensor("wv", [C, HPC * DH], FP16, kind="ExternalInput")
    wout_d = nc.dram_tensor("wout", [HPC * DH, C], FP16, kind="ExternalInput")
    out_d = nc.dram_tensor("out", [T, C], F32, kind="ExternalOutput")

    xt = xt_d.ap().rearrange("(ko p) t -> p ko t", p=128)
    wqk = wqk_d.ap().rearrange("(ko p) m -> p ko m", p=128)
    wv = wv_d.ap().rearrange("(ko p) m -> p ko m", p=128)
    wout = wout_d.ap().rearrange("(h p) c -> p h c", p=128)
    out = out_d.ap()

    with tile.TileContext(nc) as tc:
        from contextlib import ExitStack

        with ExitStack() as top:
            qk_pool = top.enter_context(tc.tile_pool(name="qk", bufs=1))
            vp_pool = top.enter_context(tc.tile_pool(name="vp", bufs=NT))
            wout_pool = top.enter_context(tc.tile_pool(name="wout", bufs=1))
            const_pool = top.enter_context(tc.tile_pool(name="const", bufs=1))
            wqk_pool = top.enter_context(tc.tile_pool(name="wqk", bufs=1))
            wv_pool = top.enter_context(tc.tile_pool(name="wv", bufs=1))
            slab_pool = top.enter_context(tc.tile_pool(name="slab", bufs=2))
            at_pool = top.enter_context(tc.tile_pool(name="at", bufs=12))
            acc_pool = top.enter_context(tc.tile_pool(name="acc", bufs=2))
            rec_pool = top.enter_context(tc.tile_pool(name="rec", bufs=3))
            aot_pool = top.enter_context(tc.tile_pool(name="aot", bufs=3))
            oc_pool = top.enter_context(tc.tile_pool(name="oc", bufs=6))
            # PSUM: 8 banks = 4 (qk-proj chains + scores) + 2 (v-proj
            # chains + out-proj + denominators) + 2 (AV accumulators)
            ps_big = top.enter_context(tc.tile_pool(name="ps_big", bufs=4, space="PSUM"))
            ps_aux = top.enter_context(tc.tile_pool(name="ps_aux", bufs=2, space="PSUM"))
            psd_o = top.enter_context(tc.tile_pool(name="psd_o", bufs=2, space="PSUM"))

            # q^T/k^T resident: row co<4 = q head co, co>=4 = k head co-4,
            # [d on partitions, t]
            qkT = qk_pool.tile([128, 2 * HPC, T], FP16)
            # v resident, one tile per t-tile: [tk within tile, (head, d) flat]
            vp = [vp_pool.tile([128, HPC * DH], FP16, name=f"vp{j}", tag="vp")
                  for j in range(NT)]
            wout_sb = wout_pool.tile([128, HPC, C], FP16)
            ones_mat = const_pool.tile([128, 128], FP16)
            nc.vector.memset(ones_mat[:], 1.0)
            wqk_sb = wqk_pool.tile([128, KO, 2 * HPC * DH], FP16)
            wv_sb = wv_pool.tile([128, KO, HPC * DH], FP16)

            def slab_dma(s):
                # 4-ko-group transfers: one trigger instruction costs ~600ns
                # of queue time, so batch them; weight loads ride the scalar
                # engine's queue in parallel with x loads on sync
                slab = slab_pool.tile([128, KO, SLAB], FP16, tag="slab",
                                      name=f"slab{s}")
                for ko in range(KO):
                    if s == 0:
                        # interleave weight/x loads so the first psum chain
                        # unblocks per-ko
                        nc.sync.dma_start(wqk_sb[:, ko], wqk[:, ko])
                    nc.sync.dma_start(slab[:, ko], xt[:, ko, s * SLAB:(s + 1) * SLAB])
                if s == 0:
                    for ko in range(KO):
                        nc.sync.dma_start(wv_sb[:, ko], wv[:, ko])
                return slab

            def qk_mms_komajor(s, slab, cos):
                # 4 psum chains in parallel, ko-major: each arriving ko of
                # wqk/x DMA immediately feeds 4 matmuls, so the cold-start
                # chains stream at DMA rate instead of serializing per-chain
                pss = [ps_big.tile([128, SLAB], F32, tag="ps512", name="ps_a")
                       for _ in cos]
                for ko in range(KO):
                    for ps, co in zip(pss, cos):
                        nc.tensor.matmul(
                            ps[:], wqk_sb[:, ko, co * 128:(co + 1) * 128],
                            slab[:, ko], start=(ko == 0), stop=(ko == KO - 1),
                        )
                for ps, co in zip(pss, cos):
                    nc.vector.tensor_copy(qkT[:, co, s * SLAB:(s + 1) * SLAB], ps[:])

            def qk_chain_mms(s, slab, cos):
                """q/k projection matmul chains of slab s for rows `cos`,
                yielding per matmul so they can be pumped as PE filler."""
                for co in cos:
                    ps = ps_big.tile([128, SLAB], F32, tag="ps512", name="ps_a")
                    for ko in range(KO):
                        nc.tensor.matmul(
                            ps[:], wqk_sb[:, ko, co * 128:(co + 1) * 128],
                            slab[:, ko], start=(ko == 0), stop=(ko == KO - 1),
                        )
                        yield
                    nc.vector.tensor_copy(qkT[:, co, s * SLAB:(s + 1) * SLAB], ps[:])

            def v_chain_mms(s, slab):
                for tt in range(SLAB // 128):
                    ps = ps_aux.tile([128, HPC * DH], F32, tag="psf", name="ps_v")
                    for ko in range(KO):
                        nc.tensor.matmul(
                            ps[:], slab[:, ko, tt * 128:(tt + 1) * 128],
                            wv_sb[:, ko], start=(ko == 0), stop=(ko == KO - 1),
                        )
                        yield
                    nc.vector.tensor_copy(vp[s * (SLAB // 128) + tt][:], ps[:])

            def slab_mms(s, slab):
                yield from qk_chain_mms(s, slab, range(2 * HPC))
                yield from v_chain_mms(s, slab)

            def proj_mms(bb, aot_bb):
                """Out projection of block bb, yielding per matmul."""
                for il in range(4):
                    for cb in range(4):
                        ps_f = ps_aux.tile([128, BW], F32, tag="psf", name="ps_f")
                        for hh in range(HPC):
                            nc.tensor.matmul(
                                ps_f[:], aot_bb[:, hh, il * 128:(il + 1) * 128],
                                wout_sb[:, hh, cb * BW:(cb + 1) * BW],
                                start=(hh == 0), stop=(hh == HPC - 1),
                            )
                            yield
                        oc = oc_pool.tile([128, BW], F32)
                        # alternate the psum drain between DVE and ACT so
                        # neither serializes the out-proj pipeline
                        if cb % 2 == 0:
                            nc.vector.tensor_copy(oc[:], ps_f[:])
                        else:
                            nc.scalar.activation(
                                oc[:], ps_f[:],
                                mybir.ActivationFunctionType.Copy)
                        nc.sync.dma_start(
                            out[(4 * bb + il) * 128:(4 * bb + il + 1) * 128,
                                cb * BW:(cb + 1) * BW], oc[:],
                        )

            # prologue: slab 0 runs un-pumped (nothing to hide it under),
            # ko-major across ALL 8 chains -- the attention psum pools are
            # idle here, so borrow their banks; 8 matmuls per arriving ko
            # (~1.7us) matches the DMA delivery rate (~1.6us/ko), keeping
            # the PE streaming through the whole cold start
            slab = slab_dma(0)
            cos8 = (0, 4, 1, 5, 2, 6, 3, 7)
            pss8 = ([ps_big.tile([128, SLAB], F32, tag="ps512", name="ps_a")
                     for _ in range(4)]
                    + [psd_o.tile([128, SLAB], F32, tag="ps_o", name="ps_ow")
                       for _ in range(2)]
                    + [ps_aux.tile([128, SLAB], F32, tag="psf", name="ps_fw")
                       for _ in range(2)])
            for ko in range(KO):
                for ps, co in zip(pss8, cos8):
                    nc.tensor.matmul(
                        ps[:], wqk_sb[:, ko, co * 128:(co + 1) * 128],
                        slab[:, ko], start=(ko == 0), stop=(ko == KO - 1),
                    )
            for ps, co in zip(pss8, cos8):
                nc.vector.tensor_copy(qkT[:, co, 0:SLAB], ps[:])
            for _ in v_chain_mms(0, slab):
                pass

            def head_tail(h, pend, ps_o_h, acc_h, js_b, aot_b):
                # final AV matmul, denominator reduce+broadcast, reciprocal,
                # normalize -- one head's post-j-loop work
                pat, ps0, pidx = pend
                nc.tensor.matmul(
                    ps_o_h[:, ps0:],
                    vp[js_b[pidx][0]][:, h * DH:(h + 1) * DH],
                    pat[:, ps0:], start=(pidx == 0), stop=True)
                ps_n = ps_aux.tile([128, BW], F32, tag="psf", name="ps_n")
                nc.tensor.matmul(ps_n[:], ones_mat[:], acc_h[:],
                                 start=True, stop=True)
                rec = rec_pool.tile([128, BW], F32)
                nc.vector.reciprocal_approx_fast(rec[:], ps_n[:])
                nc.vector.tensor_mul(aot_b[:, h], ps_o_h[:], rec[:])

            deferred = []
            aots = []
            for b in range(NB):
                if b == 0:
                    for hh in range(HPC):
                        nc.sync.dma_start(wout_sb[:, hh], wout[:, hh])
                pumps = []
                n_items = 0
                if b + 1 < NSLAB:
                    nxt = slab_dma(b + 1)
                    pumps.append(slab_mms(b + 1, nxt))
                    n_items += 192
                # the last block is exp-bound and has no slab left to pump,
                # so block 2's out projection is held back for it (and block
                # 1's rides along there too)
                if b == 1:
                    pumps.append(proj_mms(0, aots[0]))
                    n_items += 64
                elif b == NB - 1:
                    pumps.append(proj_mms(1, aots[1]))
                    pumps.append(proj_mms(2, aots[2]))
                    n_items += 128
                pump = itertools.chain(*pumps)
                nj = 4 * b + 4
                steps = 2 * nj
                per_step = -(-n_items // steps)

                aot = aot_pool.tile([128, HPC, BW], FP16)
                aots.append(aot)
                # j order: off-diagonal full-width tiles first (their k/v
                # slabs have been resident for a while, whereas the diagonal
                # group depends on slab b's just-pumped casts), then the
                # diagonal group trimmed to columns >= the diagonal.  First
                # j is always full width -> clean psum start.
                js = ([(j, 0) for j in range(4 * b)]
                      + [(4 * b + r, 128 * r) for r in range(4)])
                for hp in range(2):
                    heads = (2 * hp, 2 * hp + 1)
                    ps_o = {h: psd_o.tile([128, BW], F32, tag="ps_o",
                                          name=f"ps_o{h}")
                            for h in heads}
                    acc = {h: acc_pool.tile([128, BW], FP16, tag="acc",
                                            name=f"acc{h}")
                           for h in heads}
                    pending = {}
                    for idx, (j, s0) in enumerate(js):
                        for h in heads:
                            ps_s = ps_big.tile([128, BW], F32, tag="ps512",
                                               name="ps_s")
                            nc.tensor.matmul(
                                ps_s[:, s0:],
                                qkT[:, HPC + h, j * 128:(j + 1) * 128],
                                qkT[:, h, b * BW + s0:(b + 1) * BW],
                                start=True, stop=True)
                            at = at_pool.tile([128, BW], FP16)
                            nc.scalar.activation(
                                at[:, s0:], ps_s[:, s0:],
                                mybir.ActivationFunctionType.Exp, scale=SCALE,
                            )
                            if j >= 4 * b:
                                # causal mask: zero attnT where tk > tq
                                nc.gpsimd.affine_select(
                                    out=at[:, s0:], in_=at[:, s0:],
                                    pattern=[[1, BW - s0]],
                                    compare_op=mybir.AluOpType.is_ge, fill=0.0,
                                    base=s0 - 128 * (j - 4 * b),
                                    channel_multiplier=-1,
                                )
                            # softmax denominator: accumulate exp tiles on
                            # the DVE (fp16 -> 2x mode)
                            if idx == 0:
                                nc.vector.tensor_copy(acc[h][:], at[:])
                            else:
                                with nc.allow_low_precision("fp16 denom acc"):
                                    nc.vector.tensor_add(
                                        acc[h][:, s0:], acc[h][:, s0:],
                                        at[:, s0:])
                            prev = pending.get(h)
                            pending[h] = (at, s0, idx)
                            if prev is not None:
                                pat, ps0, pidx = prev
                                nc.tensor.matmul(
                                    ps_o[h][:, ps0:],
                                    vp[js[pidx][0]][:, h * DH:(h + 1) * DH],
                                    pat[:, ps0:],
                                    start=(pidx == 0), stop=False)
                        # previous pair's deferred tail work, then
                        # dependency-free PE filler while ACT runs exp
                        if deferred:
                            args = deferred.pop(0)
                            head_tail(*args)
                        for _ in range(per_step):
                            if next(pump, "END") == "END":
                                break
                    # each pair's tails are deferred into the NEXT pair's /
                    # block's j-steps so no transition serializes on the
                    # denominator -> reciprocal -> normalize chain
                    for args in deferred:
                        head_tail(*args)
                    deferred = [(h, pending[h], ps_o[h], acc[h], js, aot)
                                for h in heads]
                for _ in pump:
                    pass
            for args in deferred:
                head_tail(*args)
            for _ in proj_mms(NB - 1, aots[NB - 1]):
                pass

    nc.compile()
    return nc


_NC = None


def _get_nc():
    global _NC
    if _NC is None:
        _NC = build_nc()
    return _NC


def kernel(x, mask, Wqkv, Wout, _trace=False):
    assert x.shape == (B, T, C) and Wqkv.shape == (C, 3 * C) and Wout.shape == (C, C)
    nc = _get_nc()

    xt = [np.ascontiguousarray(np.asarray(x[b], dtype=np.float32).T).astype(np.float16)
          for b in range(B)]
    in_maps = []
    for c in range(8):
        b, g = c // 4, c % 4
        h0 = g * HPC * DH          # column offset of this core's heads
        wqk_c = np.ascontiguousarray(
            np.concatenate([Wqkv[:, h0:h0 + HPC * DH],
                            Wqkv[:, C + h0:C + h0 + HPC * DH]],
                           axis=1)).astype(np.float16)
        wv_c = np.ascontiguousarray(
            Wqkv[:, 2 * C + h0:2 * C + h0 + HPC * DH]).astype(np.float16)
        wout_c = np.ascontiguousarray(Wout[h0:h0 + HPC * DH, :]).astype(np.float16)
        in_maps.append({"xt": xt[b], "wqk": wqk_c, "wv": wv_c, "wout": wout_c})

    kwargs = {}
    if _trace:
        import os
        kwargs = dict(trace=True, tmpdir=os.environ.get("KERNEL_TRACE_DIR"))
    res = run_bass_kernel_spmd(nc, in_maps, core_ids=list(range(8)), **kwargs)

    outs = np.zeros((B, T, C), dtype=np.float64)
    for c in range(8):
        outs[c // 4] += res.results[c]["out"].astype(np.float64)
    result = outs.astype(np.float32)
    if _trace:
        return result, res
    return result


# revision 38
# speedup vs baseline: 1.1842x; 1.1842x over previous
"""Multi-head causal attention (B=2, T=2048, C=2048, 16 heads, fp32) on 8
Trainium2 NeuronCores.

Sharding: data-parallel over batch (2) x tensor-parallel over heads
(4 heads/core).  Core c handles batch c//4, heads 4*(c%4)..4*(c%4)+3.
Each core computes q/k/v projections for its heads, causal softmax
attention, and a partial output projection (its heads' rows of Wout);
the host sums the 4 partials per batch.

v4 design notes:
  * all matmul operands in fp16 (PSUM accumulation stays fp32).  fp16's
    10-bit mantissa keeps max-rel error ~5e-4 (measured, vs 2e-2 gate);
    every value fits fp16 range (|scores| <~ 8, exp <~ 1.2e3, denom
    <~ 9e3 << 65504).  Halves DMA + SBUF so q^T/k^T stay RESIDENT in
    SBUF -- no DRAM bounce, no attention-phase input DMA.  fp16 matmuls
    run 1 PE cycle/row at ANY free width, and fp16 tensor ops get the
    DVE 2x mode.  (fp8 was simulated: 2x PE on paper, but even
    out-proj-only fp8e4m3 measures 3.8e-2 max-rel -- over the gate.)
  * softmax denominators don't burn a PE matmul per k-tile: exp tiles
    are accumulated on the DVE into an fp16 acc tile, and ONE
    ones-matmul per (head, block) partition-reduces + broadcasts it.
  * causal diagonal tiles are trimmed: scores matmul / exp / AV matmul
    only cover columns >= the diagonal.  Each (head, block) visits its
    diagonal group first -- the first AV matmul is full width (clean
    PSUM start) -- then the off-diagonal tiles.
  * attention runs TWO heads interleaved per j-step (independent
    score->exp->mask->AV chains, pending-delayed one step) so the
    in-order engine queues always hold independent work.
  * the whole kernel is software-pipelined: projection slab s+1's
    matmuls and the out-projection of block b-1 are PUMPED between the
    j-steps of attention block b as dependency-free PE filler while ACT
    chews the exp backlog.  PSUM pools are shared across phases (qk
    chains + scores; v chains + out-proj + denominators) to fit 8 banks.
  * out-proj PSUM drains ride the DVE; reciprocals use
    reciprocal_approx_fast (~5x cheaper, 18 good bits).
"""

import itertools

import numpy as np

import concourse.bass as bass
import concourse.tile as tile
from concourse import bacc, mybir
from concourse.bass_utils import run_bass_kernel_spmd

B, T, C = 2, 2048, 2048
H, DH = 16, 128
HPC = 4            # heads per core
KO = C // 128      # 16 contraction tiles
NSLAB = 4          # 512-wide t slabs in the projection phase
SLAB = T // NSLAB  # 512
NB = 4             # 512-wide tq blocks in attention
BW = T // NB       # 512
NT = T // 128      # 16 t tiles
SCALE = DH ** -0.5
F32 = mybir.dt.float32
FP16 = mybir.dt.float16


def build_nc():
    nc = bacc.Bacc("TRN2", target_bir_lowering=False, debug=False, num_devices=8)
    xt_d = nc.dram_tensor("xt", [C, T], FP16, kind="ExternalInput")
    wqk_d = nc.dram_tensor("wqk", [C, 2 * HPC * DH], FP16, kind="ExternalInput")
    wv_d = nc.dram_tensor("wv", [C, HPC * DH], FP16, kind="ExternalInput")
    wout_d = nc.dram_tensor("wout", [HPC * DH, C], FP16, kind="ExternalInput")
    out_d = nc.dram_tensor("out", [T, C], F32, kind="ExternalOutput")

    xt = xt_d.ap().rearrange("(ko p) t -> p ko t", p=128)
    wqk = wqk_d.ap().rearrange("(ko p) m -> p ko m", p=128)
    wv = wv_d.ap().rearrange("(ko p) m -> p ko m", p=128)
    wout = wout_d.ap().rearrange("(h p) c -> p h c", p=128)
    out = out_d.ap()

    with tile.TileContext(nc) as tc:
        from contextlib import ExitStack

        with ExitStack() as top:
            qk_pool = top.enter_context(tc.tile_pool(name="qk", bufs=1))
            vp_pool = top.enter_context(tc.tile_pool(name="vp", bufs=NT))
            wout_pool = top.enter_context(tc.tile_pool(name="wout", bufs=1))
            const_pool = top.enter_context(tc.tile_pool(name="const", bufs=1))
            wqk_pool = top.enter_context(tc.tile_pool(name="wqk", bufs=1))
            wv_pool = top.enter_context(tc.tile_pool(name="wv", bufs=1))
            slab_pool = top.enter_context(tc.tile_pool(name="slab", bufs=2))
            at_pool = top.enter_context(tc.tile_pool(name="at", bufs=12))
            acc_pool = top.enter_context(tc.tile_pool(name="acc", bufs=2))
            rec_pool = top.enter_context(tc.tile_pool(name="rec", bufs=3))
            aot_pool = top.enter_context(tc.tile_pool(name="aot", bufs=3))
            oc_pool = top.enter_context(tc.tile_pool(name="oc", bufs=6))
            # PSUM: 8 banks = 4 (qk-proj chains + scores) + 2 (v-proj
            # chains + out-proj + denominators) + 2 (AV accumulators)
            ps_big = top.enter_context(tc.tile_pool(name="ps_big", bufs=4, space="PSUM"))
            ps_aux = top.enter_context(tc.tile_pool(name="ps_aux", bufs=2, space="PSUM"))
            psd_o = top.enter_context(tc.tile_pool(name="psd_o", bufs=2, space="PSUM"))

            # q^T/k^T resident: row co<4 = q head co, co>=4 = k head co-4,
            # [d on partitions, t]
            qkT = qk_pool.tile([128, 2 * HPC, T], FP16)
            # v resident, one tile per t-tile: [tk within tile, (head, d) flat]
            vp = [vp_pool.tile([128, HPC * DH], FP16, name=f"vp{j}", tag="vp")
                  for j in range(NT)]
            wout_sb = wout_pool.tile([128, HPC, C], FP16)
            ones_mat = const_pool.tile([128, 128], FP16)
            nc.vector.memset(ones_mat[:], 1.0)
            wqk_sb = wqk_pool.tile([128, KO, 2 * HPC * DH], FP16)
            wv_sb = wv_pool.tile([128, KO, HPC * DH], FP16)

            def slab_dma(s):
                # 4-ko-group transfers: one trigger instruction costs ~600ns
                # of queue time, so batch them; weight loads ride the scalar
                # engine's queue in parallel with x loads on sync
                slab = slab_pool.tile([128, KO, SLAB], FP16, tag="slab",
                                      name=f"slab{s}")
                for ko in range(KO):
                    if s == 0:
                        # interleave weight/x loads so the first psum chain
                        # unblocks per-ko
                        nc.sync.dma_start(wqk_sb[:, ko], wqk[:, ko])
                    nc.sync.dma_start(slab[:, ko], xt[:, ko, s * SLAB:(s + 1) * SLAB])
                if s == 0:
                    for ko in range(KO):
                        nc.sync.dma_start(wv_sb[:, ko], wv[:, ko])
                return slab

            def qk_mms_komajor(s, slab, cos):
                # 4 psum chains in parallel, ko-major: each arriving ko of
                # wqk/x DMA immediately feeds 4 matmuls, so the cold-start
                # chains stream at DMA rate instead of serializing per-chain
                pss = [ps_big.tile([128, SLAB], F32, tag="ps512", name="ps_a")
                       for _ in cos]
                for ko in range(KO):
                    for ps, co in zip(pss, cos):
                        nc.tensor.matmul(
                            ps[:], wqk_sb[:, ko, co * 128:(co + 1) * 128],
                            slab[:, ko], start=(ko == 0), stop=(ko == KO - 1),
                        )
                for ps, co in zip(pss, cos):
                    nc.vector.tensor_copy(qkT[:, co, s * SLAB:(s + 1) * SLAB], ps[:])

            def qk_chain_mms(s, slab, cos):
                """q/k projection matmul chains of slab s for rows `cos`,
                yielding per matmul so they can be pumped as PE filler."""
                for co in cos:
                    ps = ps_big.tile([128, SLAB], F32, tag="ps512", name="ps_a")
                    for ko in range(KO):
                        nc.tensor.matmul(
                            ps[:], wqk_sb[:, ko, co * 128:(co + 1) * 128],
                            slab[:, ko], start=(ko == 0), stop=(ko == KO - 1),
                        )
                        yield
                    nc.vector.tensor_copy(qkT[:, co, s * SLAB:(s + 1) * SLAB], ps[:])

            def v_chain_mms(s, slab):
                for tt in range(SLAB // 128):
                    ps = ps_aux.tile([128, HPC * DH], F32, tag="psf", name="ps_v")
                    for ko in range(KO):
                        nc.tensor.matmul(
                            ps[:], slab[:, ko, tt * 128:(tt + 1) * 128],
                            wv_sb[:, ko], start=(ko == 0), stop=(ko == KO - 1),
                        )
                        yield
                    nc.vector.tensor_copy(vp[s * (SLAB // 128) + tt][:], ps[:])

            def slab_mms(s, slab):
                yield from qk_chain_mms(s, slab, range(2 * HPC))
                yield from v_chain_mms(s, slab)

            def proj_mms(bb, aot_bb):
                """Out projection of block bb, yielding per matmul."""
                for il in range(4):
                    for cb in range(4):
                        ps_f = ps_aux.tile([128, BW], F32, tag="psf", name="ps_f")
                        for hh in range(HPC):
                            nc.tensor.matmul(
                                ps_f[:], aot_bb[:, hh, il * 128:(il + 1) * 128],
                                wout_sb[:, hh, cb * BW:(cb + 1) * BW],
                                start=(hh == 0), stop=(hh == HPC - 1),
                            )
                            yield
                        oc = oc_pool.tile([128, BW], F32)
                        # alternate the psum drain between DVE and ACT so
                        # neither serializes the out-proj pipeline
                        if cb % 2 == 0:
                            nc.vector.tensor_copy(oc[:], ps_f[:])
                        else:
                            nc.scalar.activation(
                                oc[:], ps_f[:],
                                mybir.ActivationFunctionType.Copy)
                        nc.sync.dma_start(
                            out[(4 * bb + il) * 128:(4 * bb + il + 1) * 128,
                                cb * BW:(cb + 1) * BW], oc[:],
                        )

            # prologue: slab 0 runs un-pumped (nothing to hide it under),
            # ko-major so the PE streams at DMA arrival rate
            slab = slab_dma(0)
            qk_mms_komajor(0, slab, (0, 4, 1, 5))
            qk_mms_komajor(0, slab, (2, 6, 3, 7))
            for _ in v_chain_mms(0, slab):
                pass

            def head_tail(h, pend, ps_o_h, acc_h, js_b, aot_b):
                # final AV matmul, denominator reduce+broadcast, reciprocal,
                # normalize -- one head's post-j-loop work
                pat, ps0, pidx = pend
                nc.tensor.matmul(
                    ps_o_h[:, ps0:],
                    vp[js_b[pidx][0]][:, h * DH:(h + 1) * DH],
                    pat[:, ps0:], start=(pidx == 0), stop=True)
                ps_n = ps_aux.tile([128, BW], F32, tag="psf", name="ps_n")
                nc.tensor.matmul(ps_n[:], ones_mat[:], acc_h[:],
                                 start=True, stop=True)
                rec = rec_pool.tile([128, BW], F32)
                nc.vector.reciprocal_approx_fast(rec[:], ps_n[:])
                nc.vector.tensor_mul(aot_b[:, h], ps_o_h[:], rec[:])

            deferred = []
            aots = []
            for b in range(NB):
                if b == 0:
                    for hh in range(HPC):
                        nc.sync.dma_start(wout_sb[:, hh], wout[:, hh])
                pumps = []
                n_items = 0
                if b + 1 < NSLAB:
                    nxt = slab_dma(b + 1)
                    pumps.append(slab_mms(b + 1, nxt))
                    n_items += 192
                # the last block is exp-bound and has no slab left to pump,
                # so block 2's out projection is held back for it (and block
                # 1's rides along there too)
                if b == 1:
                    pumps.append(proj_mms(0, aots[0]))
                    n_items += 64
                elif b == NB - 1:
                    pumps.append(proj_mms(1, aots[1]))
                    pumps.append(proj_mms(2, aots[2]))
                    n_items += 128
                pump = itertools.chain(*pumps)
                nj = 4 * b + 4
                steps = 2 * nj
                per_step = -(-n_items // steps)

                aot = aot_pool.tile([128, HPC, BW], FP16)
                aots.append(aot)
                # j order: off-diagonal full-width tiles first (their k/v
                # slabs have been resident for a while, whereas the diagonal
                # group depends on slab b's just-pumped casts), then the
                # diagonal group trimmed to columns >= the diagonal.  First
                # j is always full width -> clean psum start.
                js = ([(j, 0) for j in range(4 * b)]
                      + [(4 * b + r, 128 * r) for r in range(4)])
                for hp in range(2):
                    heads = (2 * hp, 2 * hp + 1)
                    ps_o = {h: psd_o.tile([128, BW], F32, tag="ps_o",
                                          name=f"ps_o{h}")
                            for h in heads}
                    acc = {h: acc_pool.tile([128, BW], FP16, tag="acc",
                                            name=f"acc{h}")
                           for h in heads}
                    pending = {}
                    for idx, (j, s0) in enumerate(js):
                        for h in heads:
                            ps_s = ps_big.tile([128, BW], F32, tag="ps512",
                                               name="ps_s")
                            nc.tensor.matmul(
                                ps_s[:, s0:],
                                qkT[:, HPC + h, j * 128:(j + 1) * 128],
                                qkT[:, h, b * BW + s0:(b + 1) * BW],
                                start=True, stop=True)
                            at = at_pool.tile([128, BW], FP16)
                            nc.scalar.activation(
                                at[:, s0:], ps_s[:, s0:],
                                mybir.ActivationFunctionType.Exp, scale=SCALE,
                            )
                            if j >= 4 * b:
                                # causal mask: zero attnT where tk > tq
                                nc.gpsimd.affine_select(
                                    out=at[:, s0:], in_=at[:, s0:],
                                    pattern=[[1, BW - s0]],
                                    compare_op=mybir.AluOpType.is_ge, fill=0.0,
                                    base=s0 - 128 * (j - 4 * b),
                                    channel_multiplier=-1,
                                )
                            # softmax denominator: accumulate exp tiles on
                            # the DVE (fp16 -> 2x mode)
                            if idx == 0:
                                nc.vector.tensor_copy(acc[h][:], at[:])
                            else:
                                with nc.allow_low_precision("fp16 denom acc"):
                                    nc.vector.tensor_add(
                                        acc[h][:, s0:], acc[h][:, s0:],
                                        at[:, s0:])
                            prev = pending.get(h)
                            pending[h] = (at, s0, idx)
                            if prev is not None:
                                pat, ps0, pidx = prev
                                nc.tensor.matmul(
                                    ps_o[h][:, ps0:],
                                    vp[js[pidx][0]][:, h * DH:(h + 1) * DH],
                                    pat[:, ps0:],
                                    start=(pidx == 0), stop=False)
                        # previous pair's deferred tail work, then
                        # dependency-free PE filler while ACT runs exp
                        if deferred:
                            args = deferred.pop(0)
                            head_tail(*args)
                        for _ in range(per_step):
                            if next(pump, "END") == "END":
                                break
                    # each pair's tails are deferred into the NEXT pair's /
                    # block's j-steps so no transition serializes on the
                    # denominator -> reciprocal -> normalize chain
                    for args in deferred:
                        head_tail(*args)
                    deferred = [(h, pending[h], ps_o[h], acc[h], js, aot)
                                for h in heads]
                for _ in pump:
                    pass
            for args in deferred:
                head_tail(*args)
            for _ in proj_mms(NB - 1, aots[NB - 1]):
                pass

    nc.compile()
    return nc


_NC = None


def _get_nc():
    global _NC
    if _NC is None:
        _NC = build_nc()
    return _NC


def kernel(x, mask, Wqkv, Wout, _trace=False):
    assert x.shape == (B, T, C) and Wqkv.shape == (C, 3 * C) and Wout.shape == (C, C)
    nc = _get_nc()

    xt = [np.ascontiguousarray(np.asarray(x[b], dtype=np.float32).T).astype(np.float16)
          for b in range(B)]
    in_maps = []
    for c in range(8):
        b, g = c // 4, c % 4
        h0 = g * HPC * DH          # column offset of this core's heads
        wqk_c = np.ascontiguousarray(
            np.concatenate([Wqkv[:, h0:h0 + HPC * DH],
                            Wqkv[:, C + h0:C + h0 + HPC * DH]],
                           axis=1)).astype(np.float16)
        wv_c = np.ascontiguousarray(
            Wqkv[:, 2 * C + h0:2 * C + h0 + HPC * DH]).astype(np.float16)
        wout_c = np.ascontiguousarray(Wout[h0:h0 + HPC * DH, :]).astype(np.float16)
        in_maps.append({"xt": xt[b], "wqk": wqk_c, "wv": wv_c, "wout": wout_c})

    kwargs = {}
    if _trace:
        import os
        kwargs = dict(trace=True, tmpdir=os.environ.get("KERNEL_TRACE_DIR"))
    res = run_bass_kernel_spmd(nc, in_maps, core_ids=list(range(8)), **kwargs)

    outs = np.zeros((B, T, C), dtype=np.float64)
    for c in range(8):
        outs[c // 4] += res.results[c]["out"].astype(np.float64)
    result = outs.astype(np.float32)
    if _trace:
        return result, res
    return result


# revision 39
# speedup vs baseline: 1.1863x; 1.0017x over previous
"""Multi-head causal attention (B=2, T=2048, C=2048, 16 heads, fp32) on 8
Trainium2 NeuronCores.

Sharding: data-parallel over batch (2) x tensor-parallel over heads
(4 heads/core).  Core c handles batch c//4, heads 4*(c%4)..4*(c%4)+3.
Each core computes q/k/v projections for its heads, causal softmax
attention, and a partial output projection (its heads' rows of Wout);
the host sums the 4 partials per batch.

v4 design notes:
  * all matmul operands in fp16 (PSUM accumulation stays fp32).  fp16's
    10-bit mantissa keeps max-rel error ~5e-4 (measured, vs 2e-2 gate);
    every value fits fp16 range (|scores| <~ 8, exp <~ 1.2e3, denom
    <~ 9e3 << 65504).  Halves DMA + SBUF so q^T/k^T stay RESIDENT in
    SBUF -- no DRAM bounce, no attention-phase input DMA.  fp16 matmuls
    run 1 PE cycle/row at ANY free width, and fp16 tensor ops get the
    DVE 2x mode.  (fp8 was simulated: 2x PE on paper, but even
    out-proj-only fp8e4m3 measures 3.8e-2 max-rel -- over the gate.)
  * softmax denominators don't burn a PE matmul per k-tile: exp tiles
    are accumulated on the DVE into an fp16 acc tile, and ONE
    ones-matmul per (head, block) partition-reduces + broadcasts it.
  * causal diagonal tiles are trimmed: scores matmul / exp / AV matmul
    only cover columns >= the diagonal.  Each (head, block) visits its
    diagonal group first -- the first AV matmul is full width (clean
    PSUM start) -- then the off-diagonal tiles.
  * attention runs TWO heads interleaved per j-step (independent
    score->exp->mask->AV chains, pending-delayed one step) so the
    in-order engine queues always hold independent work.
  * the whole kernel is software-pipelined: projection slab s+1's
    matmuls and the out-projection of block b-1 are PUMPED between the
    j-steps of attention block b as dependency-free PE filler while ACT
    chews the exp backlog.  PSUM pools are shared across phases (qk
    chains + scores; v chains + out-proj + denominators) to fit 8 banks.
  * out-proj PSUM drains ride the DVE; reciprocals use
    reciprocal_approx_fast (~5x cheaper, 18 good bits).
"""

import itertools

import numpy as np

import concourse.bass as bass
import concourse.tile as tile
from concourse import bacc, mybir
from concourse.bass_utils import run_bass_kernel_spmd

B, T, C = 2, 2048, 2048
H, DH = 16, 128
HPC = 4            # heads per core
KO = C // 128      # 16 contraction tiles
NSLAB = 4          # 512-wide t slabs in the projection phase
SLAB = T // NSLAB  # 512
NB = 4             # 512-wide tq blocks in attention
BW = T // NB       # 512
NT = T // 128      # 16 t tiles
SCALE = DH ** -0.5
F32 = mybir.dt.float32
FP16 = mybir.dt.float16


def build_nc():
    nc = bacc.Bacc("TRN2", target_bir_lowering=False, debug=False, num_devices=8)
    xt_d = nc.dram_tensor("xt", [C, T], FP16, kind="ExternalInput")
    wqk_d = nc.dram_tensor("wqk", [C, 2 * HPC * DH], FP16, kind="ExternalInput")
    wv_d = nc.dram_tensor("wv", [C, HPC * DH], FP16, kind="ExternalInput")
    wout_d = nc.dram_tensor("wout", [HPC * DH, C], FP16, kind="ExternalInput")
    out_d = nc.dram_tensor("out", [T, C], F32, kind="ExternalOutput")

    xt = xt_d.ap().rearrange("(ko p) t -> p ko t", p=128)
    wqk = wqk_d.ap().rearrange("(ko p) m -> p ko m", p=128)
    wv = wv_d.ap().rearrange("(ko p) m -> p ko m", p=128)
    wout = wout_d.ap().rearrange("(h p) c -> p h c", p=128)
    out = out_d.ap()

    with tile.TileContext(nc) as tc:
        from contextlib import ExitStack

        with ExitStack() as top:
            qk_pool = top.enter_context(tc.tile_pool(name="qk", bufs=1))
            vp_pool = top.enter_context(tc.tile_pool(name="vp", bufs=NT))
            wout_pool = top.enter_context(tc.tile_pool(name="wout", bufs=1))
            const_pool = top.enter_context(tc.tile_pool(name="const", bufs=1))
            wqk_pool = top.enter_context(tc.tile_pool(name="wqk", bufs=1))
            wv_pool = top.enter_context(tc.tile_pool(name="wv", bufs=1))
            slab_pool = top.enter_context(tc.tile_pool(name="slab", bufs=2))
            at_pool = top.enter_context(tc.tile_pool(name="at", bufs=12))
            acc_pool = top.enter_context(tc.tile_pool(name="acc", bufs=2))
            rec_pool = top.enter_context(tc.tile_pool(name="rec", bufs=3))
            aot_pool = top.enter_context(tc.tile_pool(name="aot", bufs=3))
            oc_pool = top.enter_context(tc.tile_pool(name="oc", bufs=6))
            # PSUM: 8 banks = 4 (qk-proj chains + scores) + 2 (v-proj
            # chains + out-proj + denominators) + 2 (AV accumulators)
            ps_big = top.enter_context(tc.tile_pool(name="ps_big", bufs=4, space="PSUM"))
            ps_aux = top.enter_context(tc.tile_pool(name="ps_aux", bufs=2, space="PSUM"))
            psd_o = top.enter_context(tc.tile_pool(name="psd_o", bufs=2, space="PSUM"))

            # q^T/k^T resident: row co<4 = q head co, co>=4 = k head co-4,
            # [d on partitions, t]
            qkT = qk_pool.tile([128, 2 * HPC, T], FP16)
            # v resident, one tile per t-tile: [tk within tile, (head, d) flat]
            vp = [vp_pool.tile([128, HPC * DH], FP16, name=f"vp{j}", tag="vp")
                  for j in range(NT)]
            wout_sb = wout_pool.tile([128, HPC, C], FP16)
            ones_mat = const_pool.tile([128, 128], FP16)
            nc.vector.memset(ones_mat[:], 1.0)
            wqk_sb = wqk_pool.tile([128, KO, 2 * HPC * DH], FP16)
            wv_sb = wv_pool.tile([128, KO, HPC * DH], FP16)

            def slab_dma(s):
                # 4-ko-group transfers: one trigger instruction costs ~600ns
                # of queue time, so batch them; weight loads ride the scalar
                # engine's queue in parallel with x loads on sync
                slab = slab_pool.tile([128, KO, SLAB], FP16, tag="slab",
                                      name=f"slab{s}")
                for ko in range(KO):
                    if s == 0:
                        # interleave weight/x loads so the first psum chain
                        # unblocks per-ko
                        nc.sync.dma_start(wqk_sb[:, ko], wqk[:, ko])
                    nc.sync.dma_start(slab[:, ko], xt[:, ko, s * SLAB:(s + 1) * SLAB])
                if s == 0:
                    for ko in range(KO):
                        nc.sync.dma_start(wv_sb[:, ko], wv[:, ko])
                return slab

            def qk_mms_komajor(s, slab, cos):
                # 4 psum chains in parallel, ko-major: each arriving ko of
                # wqk/x DMA immediately feeds 4 matmuls, so the cold-start
                # chains stream at DMA rate instead of serializing per-chain
                pss = [ps_big.tile([128, SLAB], F32, tag="ps512", name="ps_a")
                       for _ in cos]
                for ko in range(KO):
                    for ps, co in zip(pss, cos):
                        nc.tensor.matmul(
                            ps[:], wqk_sb[:, ko, co * 128:(co + 1) * 128],
                            slab[:, ko], start=(ko == 0), stop=(ko == KO - 1),
                        )
                for ps, co in zip(pss, cos):
                    nc.vector.tensor_copy(qkT[:, co, s * SLAB:(s + 1) * SLAB], ps[:])

            def qk_chain_mms(s, slab, cos):
                """q/k projection matmul chains of slab s for rows `cos`,
                yielding per matmul so they can be pumped as PE filler."""
                for co in cos:
                    ps = ps_big.tile([128, SLAB], F32, tag="ps512", name="ps_a")
                    for ko in range(KO):
                        nc.tensor.matmul(
                            ps[:], wqk_sb[:, ko, co * 128:(co + 1) * 128],
                            slab[:, ko], start=(ko == 0), stop=(ko == KO - 1),
                        )
                        yield
                    nc.vector.tensor_copy(qkT[:, co, s * SLAB:(s + 1) * SLAB], ps[:])

            def v_chain_mms(s, slab):
                for tt in range(SLAB // 128):
                    ps = ps_aux.tile([128, HPC * DH], F32, tag="psf", name="ps_v")
                    for ko in range(KO):
                        nc.tensor.matmul(
                            ps[:], slab[:, ko, tt * 128:(tt + 1) * 128],
                            wv_sb[:, ko], start=(ko == 0), stop=(ko == KO - 1),
                        )
                        yield
                    nc.vector.tensor_copy(vp[s * (SLAB // 128) + tt][:], ps[:])

            def slab_mms(s, slab):
                yield from qk_chain_mms(s, slab, range(2 * HPC))
                yield from v_chain_mms(s, slab)

            def proj_mms(bb, aot_bb, act_from_il=99):
                """Out projection of block bb, yielding per matmul.
                Chunks with il >= act_from_il drain on ACT: near the kernel
                tail ACT is idle while DVE is congested with the last
                reciprocal/normalize chain."""
                for il in range(4):
                    for cb in range(4):
                        ps_f = ps_aux.tile([128, BW], F32, tag="psf", name="ps_f")
                        for hh in range(HPC):
                            nc.tensor.matmul(
                                ps_f[:], aot_bb[:, hh, il * 128:(il + 1) * 128],
                                wout_sb[:, hh, cb * BW:(cb + 1) * BW],
                                start=(hh == 0), stop=(hh == HPC - 1),
                            )
                            yield
                        oc = oc_pool.tile([128, BW], F32)
                        # alternate the psum drain between DVE and ACT so
                        # neither serializes the out-proj pipeline
                        if il < act_from_il and cb % 2 == 0:
                            nc.vector.tensor_copy(oc[:], ps_f[:])
                        else:
                            nc.scalar.activation(
                                oc[:], ps_f[:],
                                mybir.ActivationFunctionType.Copy)
                        nc.sync.dma_start(
                            out[(4 * bb + il) * 128:(4 * bb + il + 1) * 128,
                                cb * BW:(cb + 1) * BW], oc[:],
                        )

            # prologue: slab 0 runs un-pumped (nothing to hide it under),
            # ko-major so the PE streams at DMA arrival rate
            slab = slab_dma(0)
            qk_mms_komajor(0, slab, (0, 4, 1, 5))
            qk_mms_komajor(0, slab, (2, 6, 3, 7))
            for _ in v_chain_mms(0, slab):
                pass

            def head_tail(h, pend, ps_o_h, acc_h, js_b, aot_b):
                # final AV matmul, denominator reduce+broadcast, reciprocal,
                # normalize -- one head's post-j-loop work
                pat, ps0, pidx = pend
                nc.tensor.matmul(
                    ps_o_h[:, ps0:],
                    vp[js_b[pidx][0]][:, h * DH:(h + 1) * DH],
                    pat[:, ps0:], start=(pidx == 0), stop=True)
                ps_n = ps_aux.tile([128, BW], F32, tag="psf", name="ps_n")
                nc.tensor.matmul(ps_n[:], ones_mat[:], acc_h[:],
                                 start=True, stop=True)
                rec = rec_pool.tile([128, BW], F32)
                nc.vector.reciprocal_approx_fast(rec[:], ps_n[:])
                nc.vector.tensor_mul(aot_b[:, h], ps_o_h[:], rec[:])

            deferred = []
            aots = []
            for b in range(NB):
                if b == 0:
                    for hh in range(HPC):
                        nc.sync.dma_start(wout_sb[:, hh], wout[:, hh])
                pumps = []
                n_items = 0
                if b + 1 < NSLAB:
                    nxt = slab_dma(b + 1)
                    pumps.append(slab_mms(b + 1, nxt))
                    n_items += 192
                # the last block is exp-bound and has no slab left to pump,
                # so block 2's out projection is held back for it (and block
                # 1's rides along there too)
                if b == 1:
                    pumps.append(proj_mms(0, aots[0]))
                    n_items += 64
                elif b == NB - 1:
                    pumps.append(proj_mms(1, aots[1]))
                    pumps.append(proj_mms(2, aots[2], act_from_il=2))
                    n_items += 128
                pump = itertools.chain(*pumps)
                nj = 4 * b + 4
                steps = 2 * nj
                per_step = -(-n_items // steps)

                aot = aot_pool.tile([128, HPC, BW], FP16)
                aots.append(aot)
                # j order: off-diagonal full-width tiles first (their k/v
                # slabs have been resident for a while, whereas the diagonal
                # group depends on slab b's just-pumped casts), then the
                # diagonal group trimmed to columns >= the diagonal.  First
                # j is always full width -> clean psum start.
                js = ([(j, 0) for j in range(4 * b)]
                      + [(4 * b + r, 128 * r) for r in range(4)])
                for hp in range(2):
                    heads = (2 * hp, 2 * hp + 1)
                    ps_o = {h: psd_o.tile([128, BW], F32, tag="ps_o",
                                          name=f"ps_o{h}")
                            for h in heads}
                    acc = {h: acc_pool.tile([128, BW], FP16, tag="acc",
                                            name=f"acc{h}")
                           for h in heads}
                    pending = {}
                    for idx, (j, s0) in enumerate(js):
                        for h in heads:
                            ps_s = ps_big.tile([128, BW], F32, tag="ps512",
                                               name="ps_s")
                            nc.tensor.matmul(
                                ps_s[:, s0:],
                                qkT[:, HPC + h, j * 128:(j + 1) * 128],
                                qkT[:, h, b * BW + s0:(b + 1) * BW],
                                start=True, stop=True)
                            at = at_pool.tile([128, BW], FP16)
                            nc.scalar.activation(
                                at[:, s0:], ps_s[:, s0:],
                                mybir.ActivationFunctionType.Exp, scale=SCALE,
                            )
                            if j >= 4 * b:
                                # causal mask: zero attnT where tk > tq
                                nc.gpsimd.affine_select(
                                    out=at[:, s0:], in_=at[:, s0:],
                                    pattern=[[1, BW - s0]],
                                    compare_op=mybir.AluOpType.is_ge, fill=0.0,
                                    base=s0 - 128 * (j - 4 * b),
                                    channel_multiplier=-1,
                                )
                            # softmax denominator: accumulate exp tiles on
                            # the DVE (fp16 -> 2x mode)
                            if idx == 0:
                                nc.vector.tensor_copy(acc[h][:], at[:])
                            else:
                                with nc.allow_low_precision("fp16 denom acc"):
                                    nc.vector.tensor_add(
                                        acc[h][:, s0:], acc[h][:, s0:],
                                        at[:, s0:])
                            prev = pending.get(h)
                            pending[h] = (at, s0, idx)
                            if prev is not None:
                                pat, ps0, pidx = prev
                                nc.tensor.matmul(
                                    ps_o[h][:, ps0:],
                                    vp[js[pidx][0]][:, h * DH:(h + 1) * DH],
                                    pat[:, ps0:],
                                    start=(pidx == 0), stop=False)
                        # previous pair's deferred tail work, then
                        # dependency-free PE filler while ACT runs exp
                        if deferred:
                            args = deferred.pop(0)
                            head_tail(*args)
                        for _ in range(per_step):
                            if next(pump, "END") == "END":
                                break
                    # each pair's tails are deferred into the NEXT pair's /
                    # block's j-steps so no transition serializes on the
                    # denominator -> reciprocal -> normalize chain
                    for args in deferred:
                        head_tail(*args)
                    deferred = [(h, pending[h], ps_o[h], acc[h], js, aot)
                                for h in heads]
                for _ in pump:
                    pass
            for args in deferred:
                head_tail(*args)
            for _ in proj_mms(NB - 1, aots[NB - 1], act_from_il=0):
                pass

    nc.compile()
    return nc


_NC = None


def _get_nc():
    global _NC
    if _NC is None:
        _NC = build_nc()
    return _NC


def kernel(x, mask, Wqkv, Wout, _trace=False):
    assert x.shape == (B, T, C) and Wqkv.shape == (C, 3 * C) and Wout.shape == (C, C)
    nc = _get_nc()

    xt = [np.ascontiguousarray(np.asarray(x[b], dtype=np.float32).T).astype(np.float16)
          for b in range(B)]
    in_maps = []
    for c in range(8):
        b, g = c // 4, c % 4
        h0 = g * HPC * DH          # column offset of this core's heads
        wqk_c = np.ascontiguousarray(
            np.concatenate([Wqkv[:, h0:h0 + HPC * DH],
                            Wqkv[:, C + h0:C + h0 + HPC * DH]],
                           axis=1)).astype(np.float16)
        wv_c = np.ascontiguousarray(
            Wqkv[:, 2 * C + h0:2 * C + h0 + HPC * DH]).astype(np.float16)
        wout_c = np.ascontiguousarray(Wout[h0:h0 + HPC * DH, :]).astype(np.float16)
        in_maps.append({"xt": xt[b], "wqk": wqk_c, "wv": wv_c, "wout": wout_c})

    kwargs = {}
    if _trace:
        import os
        kwargs = dict(trace=True, tmpdir=os.environ.get("KERNEL_TRACE_DIR"))
    res = run_bass_kernel_spmd(nc, in_maps, core_ids=list(range(8)), **kwargs)

    outs = np.zeros((B, T, C), dtype=np.float64)
    for c in range(8):
        outs[c // 4] += res.results[c]["out"].astype(np.float64)
    result = outs.astype(np.float32)
    if _trace:
        return result, res
    return result


# revision 40
# speedup vs baseline: 1.1893x; 1.0025x over previous
"""Multi-head causal attention (B=2, T=2048, C=2048, 16 heads, fp32) on 8
Trainium2 NeuronCores.

Sharding: data-parallel over batch (2) x tensor-parallel over heads
(4 heads/core).  Core c handles batch c//4, heads 4*(c%4)..4*(c%4)+3.
Each core computes q/k/v projections for its heads, causal softmax
attention, and a partial output projection (its heads' rows of Wout);
the host sums the 4 partials per batch.

v4 design notes:
  * all matmul operands in fp16 (PSUM accumulation stays fp32).  fp16's
    10-bit mantissa keeps max-rel error ~5e-4 (measured, vs 2e-2 gate);
    every value fits fp16 range (|scores| <~ 8, exp <~ 1.2e3, denom
    <~ 9e3 << 65504).  Halves DMA + SBUF so q^T/k^T stay RESIDENT in
    SBUF -- no DRAM bounce, no attention-phase input DMA.  fp16 matmuls
    run 1 PE cycle/row at ANY free width, and fp16 tensor ops get the
    DVE 2x mode.  (fp8 was simulated: 2x PE on paper, but even
    out-proj-only fp8e4m3 measures 3.8e-2 max-rel -- over the gate.)
  * softmax denominators don't burn a PE matmul per k-tile: exp tiles
    are accumulated on the DVE into an fp16 acc tile, and ONE
    ones-matmul per (head, block) partition-reduces + broadcasts it.
  * causal diagonal tiles are trimmed: scores matmul / exp / AV matmul
    only cover columns >= the diagonal.  Each (head, block) visits its
    diagonal group first -- the first AV matmul is full width (clean
    PSUM start) -- then the off-diagonal tiles.
  * attention runs TWO heads interleaved per j-step (independent
    score->exp->mask->AV chains, pending-delayed one step) so the
    in-order engine queues always hold independent work.
  * the whole kernel is software-pipelined: projection slab s+1's
    matmuls and the out-projection of block b-1 are PUMPED between the
    j-steps of attention block b as dependency-free PE filler while ACT
    chews the exp backlog.  PSUM pools are shared across phases (qk
    chains + scores; v chains + out-proj + denominators) to fit 8 banks.
  * out-proj PSUM drains ride the DVE; reciprocals use
    reciprocal_approx_fast (~5x cheaper, 18 good bits).
"""

import itertools

import numpy as np

import concourse.bass as bass
import concourse.tile as tile
from concourse import bacc, mybir
from concourse.bass_utils import run_bass_kernel_spmd

B, T, C = 2, 2048, 2048
H, DH = 16, 128
HPC = 4            # heads per core
KO = C // 128      # 16 contraction tiles
NSLAB = 4          # 512-wide t slabs in the projection phase
SLAB = T // NSLAB  # 512
NB = 4             # 512-wide tq blocks in attention
BW = T // NB       # 512
NT = T // 128      # 16 t tiles
SCALE = DH ** -0.5
F32 = mybir.dt.float32
FP16 = mybir.dt.float16


def build_nc():
    nc = bacc.Bacc("TRN2", target_bir_lowering=False, debug=False, num_devices=8)
    xt_d = nc.dram_tensor("xt", [C, T], FP16, kind="ExternalInput")
    wqk_d = nc.dram_tensor("wqk", [C, 2 * HPC * DH], FP16, kind="ExternalInput")
    wv_d = nc.dram_tensor("wv", [C, HPC * DH], FP16, kind="ExternalInput")
    wout_d = nc.dram_tensor("wout", [HPC * DH, C], FP16, kind="ExternalInput")
    out_d = nc.dram_tensor("out", [T, C], F32, kind="ExternalOutput")

    xt = xt_d.ap().rearrange("(ko p) t -> p ko t", p=128)
    wqk = wqk_d.ap().rearrange("(ko p) m -> p ko m", p=128)
    wv = wv_d.ap().rearrange("(ko p) m -> p ko m", p=128)
    wout = wout_d.ap().rearrange("(h p) c -> p h c", p=128)
    out = out_d.ap()

    with tile.TileContext(nc) as tc:
        from contextlib import ExitStack

        with ExitStack() as top:
            qk_pool = top.enter_context(tc.tile_pool(name="qk", bufs=1))
            vp_pool = top.enter_context(tc.tile_pool(name="vp", bufs=NT))
            wout_pool = top.enter_context(tc.tile_pool(name="wout", bufs=1))
            const_pool = top.enter_context(tc.tile_pool(name="const", bufs=1))
            wqk_pool = top.enter_context(tc.tile_pool(name="wqk", bufs=1))
            wv_pool = top.enter_context(tc.tile_pool(name="wv", bufs=1))
            slab_pool = top.enter_context(tc.tile_pool(name="slab", bufs=2))
            at_pool = top.enter_context(tc.tile_pool(name="at", bufs=12))
            acc_pool = top.enter_context(tc.tile_pool(name="acc", bufs=2))
            rec_pool = top.enter_context(tc.tile_pool(name="rec", bufs=3))
            aot_pool = top.enter_context(tc.tile_pool(name="aot", bufs=3))
            oc_pool = top.enter_context(tc.tile_pool(name="oc", bufs=6))
            # PSUM: 8 banks = 4 (qk-proj chains + scores) + 2 (v-proj
            # chains + out-proj + denominators) + 2 (AV accumulators)
            ps_big = top.enter_context(tc.tile_pool(name="ps_big", bufs=4, space="PSUM"))
            ps_aux = top.enter_context(tc.tile_pool(name="ps_aux", bufs=2, space="PSUM"))
            psd_o = top.enter_context(tc.tile_pool(name="psd_o", bufs=2, space="PSUM"))

            # q^T/k^T resident: row co<4 = q head co, co>=4 = k head co-4,
            # [d on partitions, t]
            qkT = qk_pool.tile([128, 2 * HPC, T], FP16)
            # v resident, one tile per t-tile: [tk within tile, (head, d) flat]
            vp = [vp_pool.tile([128, HPC * DH], FP16, name=f"vp{j}", tag="vp")
                  for j in range(NT)]
            wout_sb = wout_pool.tile([128, HPC, C], FP16)
            ones_mat = const_pool.tile([128, 128], FP16)
            nc.vector.memset(ones_mat[:], 1.0)
            wqk_sb = wqk_pool.tile([128, KO, 2 * HPC * DH], FP16)
            wv_sb = wv_pool.tile([128, KO, HPC * DH], FP16)

            def slab_dma(s):
                # 4-ko-group transfers: one trigger instruction costs ~600ns
                # of queue time, so batch them; weight loads ride the scalar
                # engine's queue in parallel with x loads on sync
                slab = slab_pool.tile([128, KO, SLAB], FP16, tag="slab",
                                      name=f"slab{s}")
                for ko in range(KO):
                    if s == 0:
                        # interleave weight/x loads so the first psum chain
                        # unblocks per-ko
                        nc.sync.dma_start(wqk_sb[:, ko], wqk[:, ko])
                    nc.sync.dma_start(slab[:, ko], xt[:, ko, s * SLAB:(s + 1) * SLAB])
                if s == 0:
                    for ko in range(KO):
                        nc.sync.dma_start(wv_sb[:, ko], wv[:, ko])
                return slab

            def qk_mms_komajor(s, slab, cos):
                # 4 psum chains in parallel, ko-major: each arriving ko of
                # wqk/x DMA immediately feeds 4 matmuls, so the cold-start
                # chains stream at DMA rate instead of serializing per-chain
                pss = [ps_big.tile([128, SLAB], F32, tag="ps512", name="ps_a")
                       for _ in cos]
                for ko in range(KO):
                    for ps, co in zip(pss, cos):
                        nc.tensor.matmul(
                            ps[:], wqk_sb[:, ko, co * 128:(co + 1) * 128],
                            slab[:, ko], start=(ko == 0), stop=(ko == KO - 1),
                        )
                for ps, co in zip(pss, cos):
                    nc.vector.tensor_copy(qkT[:, co, s * SLAB:(s + 1) * SLAB], ps[:])

            def qk_chain_mms(s, slab, cos):
                """q/k projection matmul chains of slab s for rows `cos`,
                yielding per matmul so they can be pumped as PE filler."""
                for co in cos:
                    ps = ps_big.tile([128, SLAB], F32, tag="ps512", name="ps_a")
                    for ko in range(KO):
                        nc.tensor.matmul(
                            ps[:], wqk_sb[:, ko, co * 128:(co + 1) * 128],
                            slab[:, ko], start=(ko == 0), stop=(ko == KO - 1),
                        )
                        yield
                    nc.vector.tensor_copy(qkT[:, co, s * SLAB:(s + 1) * SLAB], ps[:])

            def v_chain_mms(s, slab):
                for tt in range(SLAB // 128):
                    ps = ps_aux.tile([128, HPC * DH], F32, tag="psf", name="ps_v")
                    for ko in range(KO):
                        nc.tensor.matmul(
                            ps[:], slab[:, ko, tt * 128:(tt + 1) * 128],
                            wv_sb[:, ko], start=(ko == 0), stop=(ko == KO - 1),
                        )
                        yield
                    nc.vector.tensor_copy(vp[s * (SLAB // 128) + tt][:], ps[:])

            def slab_mms(s, slab):
                yield from qk_chain_mms(s, slab, range(2 * HPC))
                yield from v_chain_mms(s, slab)

            def proj_mms(bb, aot_bb, act_from_il=99):
                """Out projection of block bb, yielding per matmul.
                Chunks with il >= act_from_il drain on ACT: near the kernel
                tail ACT is idle while DVE is congested with the last
                reciprocal/normalize chain."""
                for il in range(4):
                    for cb in range(4):
                        ps_f = ps_aux.tile([128, BW], F32, tag="psf", name="ps_f")
                        for hh in range(HPC):
                            nc.tensor.matmul(
                                ps_f[:], aot_bb[:, hh, il * 128:(il + 1) * 128],
                                wout_sb[:, hh, cb * BW:(cb + 1) * BW],
                                start=(hh == 0), stop=(hh == HPC - 1),
                            )
                            yield
                        oc = oc_pool.tile([128, BW], F32)
                        # alternate the psum drain between DVE and ACT so
                        # neither serializes the out-proj pipeline
                        if il < act_from_il and cb % 2 == 0:
                            nc.vector.tensor_copy(oc[:], ps_f[:])
                        else:
                            nc.scalar.activation(
                                oc[:], ps_f[:],
                                mybir.ActivationFunctionType.Copy)
                        nc.sync.dma_start(
                            out[(4 * bb + il) * 128:(4 * bb + il + 1) * 128,
                                cb * BW:(cb + 1) * BW], oc[:],
                        )

            # prologue: slab 0 runs un-pumped (nothing to hide it under),
            # ko-major so the PE streams at DMA arrival rate
            slab = slab_dma(0)
            qk_mms_komajor(0, slab, (0, 4, 1, 5))
            qk_mms_komajor(0, slab, (2, 6, 3, 7))
            for _ in v_chain_mms(0, slab):
                pass

            def head_tail(h, pend, ps_o_h, acc_h, js_b, aot_b):
                # final AV matmul, denominator reduce+broadcast, reciprocal,
                # normalize -- one head's post-j-loop work
                pat, ps0, pidx = pend
                nc.tensor.matmul(
                    ps_o_h[:, ps0:],
                    vp[js_b[pidx][0]][:, h * DH:(h + 1) * DH],
                    pat[:, ps0:], start=(pidx == 0), stop=True)
                ps_n = ps_aux.tile([128, BW], F32, tag="psf", name="ps_n")
                nc.tensor.matmul(ps_n[:], ones_mat[:], acc_h[:],
                                 start=True, stop=True)
                rec = rec_pool.tile([128, BW], F32)
                nc.vector.reciprocal_approx_fast(rec[:], ps_n[:])
                nc.vector.tensor_mul(aot_b[:, h], ps_o_h[:], rec[:])

            deferred = []
            aots = []
            for b in range(NB):
                if b == 0:
                    for hh in range(HPC):
                        nc.sync.dma_start(wout_sb[:, hh], wout[:, hh])
                pumps = []
                n_items = 0
                if b + 1 < NSLAB:
                    nxt = slab_dma(b + 1)
                    pumps.append(slab_mms(b + 1, nxt))
                    n_items += 192
                # the last block is exp-bound and has no slab left to pump,
                # so block 2's out projection is held back for it (and block
                # 1's rides along there too)
                if b == 1:
                    pumps.append(proj_mms(0, aots[0]))
                    n_items += 64
                elif b == NB - 1:
                    pumps.append(proj_mms(1, aots[1]))
                    pumps.append(proj_mms(2, aots[2], act_from_il=2))
                    n_items += 128
                pump = itertools.chain(*pumps)
                nj = 4 * b + 4
                steps = 2 * nj
                per_step = -(-n_items // steps)

                aot = aot_pool.tile([128, HPC, BW], FP16)
                aots.append(aot)
                # j order: off-diagonal full-width tiles first (their k/v
                # slabs have been resident for a while, whereas the diagonal
                # group depends on slab b's just-pumped casts), then the
                # diagonal group trimmed to columns >= the diagonal.  First
                # j is always full width -> clean psum start.
                # ... and one off-diagonal j LAST: the final j's
                # exp->mask->AV chain gates each head's tail, and off-diag
                # tiles skip the gpsimd mask hop
                js = ([(j, 0) for j in range(1, 4 * b)]
                      + [(4 * b + r, 128 * r) for r in range(4)]
                      + ([(0, 0)] if b > 0 else []))
                for hp in range(2):
                    heads = (2 * hp, 2 * hp + 1)
                    ps_o = {h: psd_o.tile([128, BW], F32, tag="ps_o",
                                          name=f"ps_o{h}")
                            for h in heads}
                    acc = {h: acc_pool.tile([128, BW], FP16, tag="acc",
                                            name=f"acc{h}")
                           for h in heads}
                    pending = {}
                    for idx, (j, s0) in enumerate(js):
                        for h in heads:
                            ps_s = ps_big.tile([128, BW], F32, tag="ps512",
                                               name="ps_s")
                            nc.tensor.matmul(
                                ps_s[:, s0:],
                                qkT[:, HPC + h, j * 128:(j + 1) * 128],
                                qkT[:, h, b * BW + s0:(b + 1) * BW],
                                start=True, stop=True)
                            at = at_pool.tile([128, BW], FP16)
                            nc.scalar.activation(
                                at[:, s0:], ps_s[:, s0:],
                                mybir.ActivationFunctionType.Exp, scale=SCALE,
                            )
                            if j >= 4 * b:
                                # causal mask: zero attnT where tk > tq
                                nc.gpsimd.affine_select(
                                    out=at[:, s0:], in_=at[:, s0:],
                                    pattern=[[1, BW - s0]],
                                    compare_op=mybir.AluOpType.is_ge, fill=0.0,
                                    base=s0 - 128 * (j - 4 * b),
                                    channel_multiplier=-1,
                                )
                            # softmax denominator: accumulate exp tiles on
                            # the DVE (fp16 -> 2x mode)
                            if idx == 0:
                                nc.vector.tensor_copy(acc[h][:], at[:])
                            else:
                                with nc.allow_low_precision("fp16 denom acc"):
                                    nc.vector.tensor_add(
                                        acc[h][:, s0:], acc[h][:, s0:],
                                        at[:, s0:])
                            prev = pending.get(h)
                            pending[h] = (at, s0, idx)
                            if prev is not None:
                                pat, ps0, pidx = prev
                                nc.tensor.matmul(
                                    ps_o[h][:, ps0:],
                                    vp[js[pidx][0]][:, h * DH:(h + 1) * DH],
                                    pat[:, ps0:],
                                    start=(pidx == 0), stop=False)
                        # previous pair's deferred tail work, then
                        # dependency-free PE filler while ACT runs exp
                        if deferred:
                            args = deferred.pop(0)
                            head_tail(*args)
                        for _ in range(per_step):
                            if next(pump, "END") == "END":
                                break
                    # each pair's tails are deferred into the NEXT pair's /
                    # block's j-steps so no transition serializes on the
                    # denominator -> reciprocal -> normalize chain
                    for args in deferred:
                        head_tail(*args)
                    deferred = [(h, pending[h], ps_o[h], acc[h], js, aot)
                                for h in heads]
                for _ in pump:
                    pass
            for args in deferred:
                head_tail(*args)
            for _ in proj_mms(NB - 1, aots[NB - 1], act_from_il=0):
                pass

    nc.compile()
    return nc


_NC = None


def _get_nc():
    global _NC
    if _NC is None:
        _NC = build_nc()
    return _NC


def kernel(x, mask, Wqkv, Wout, _trace=False):
    assert x.shape == (B, T, C) and Wqkv.shape == (C, 3 * C) and Wout.shape == (C, C)
    nc = _get_nc()

    xt = [np.ascontiguousarray(np.asarray(x[b], dtype=np.float32).T).astype(np.float16)
          for b in range(B)]
    in_maps = []
    for c in range(8):
        b, g = c // 4, c % 4
        h0 = g * HPC * DH          # column offset of this core's heads
        wqk_c = np.ascontiguousarray(
            np.concatenate([Wqkv[:, h0:h0 + HPC * DH],
                            Wqkv[:, C + h0:C + h0 + HPC * DH]],
                           axis=1)).astype(np.float16)
        wv_c = np.ascontiguousarray(
            Wqkv[:, 2 * C + h0:2 * C + h0 + HPC * DH]).astype(np.float16)
        wout_c = np.ascontiguousarray(Wout[h0:h0 + HPC * DH, :]).astype(np.float16)
        in_maps.append({"xt": xt[b], "wqk": wqk_c, "wv": wv_c, "wout": wout_c})

    kwargs = {}
    if _trace:
        import os
        kwargs = dict(trace=True, tmpdir=os.environ.get("KERNEL_TRACE_DIR"))
    res = run_bass_kernel_spmd(nc, in_maps, core_ids=list(range(8)), **kwargs)

    outs = np.zeros((B, T, C), dtype=np.float64)
    for c in range(8):
        outs[c // 4] += res.results[c]["out"].astype(np.float64)
    result = outs.astype(np.float32)
    if _trace:
        return result, res
    return result
